# revision 40
# baseline (speedup 1.0000x reference)
"""Trainium2 Bass kernel for a single-head attention + FFN transformer block.

Math (per batch b):
  S   = kbias + Q @ K^T             (bf16 operands, fp32 PSUM accum;
                                     kbias[k] = 0 valid / -1e30 masked is the
                                     FIRST matmul of the accumulation group)
  E   = exp(S * qscale)             qscale[q] = qmask/sqrt(d); invalid q rows
                                    get scale 0 -> exp(0)=1 -> uniform softmax,
                                    exactly matching the reference's fully
                                    masked-row behaviour. E emitted in bf16.
  att = (E @ V) * recip(rowsum)     rowsum from ACT accum_out
  y   = LN1(Q + att)                fused: DVE scalar_tensor_tensor does
                                    (U * recip) + Q; LN1 apply on DVE,
                                    LN2 apply on ACT.
  H^T = relu(W1T.T @ yT + b1)
  Z   = HT.T @ W2T                  (+b2 shift cancels inside LN2 exactly)
  out = LN2(y + Z)

Sharding: data-parallel, 4 batches per core across 8 NeuronCores. Batches are
packed into per-slot rank groups by predicted cost so the shared SPMD program
can specialize each slot's S/EV work to the componentwise max of the group's
sequence lengths (extra work lands in masked regions that the kbias/qscale
trick zeroes exactly). Host pre-transposes Q/K and the FFN weights and ships
everything in bf16.
"""

import sys

sys.path.insert(0, "/opt/trn_rl_repo")

import numpy as np
import ml_dtypes

import concourse.bass as bass
import concourse.bacc as bacc
import concourse.mybir as mybir
from concourse import tile
from concourse.bass_utils import run_bass_kernel_spmd

B, QTL, KTL, D = 32, 512, 512, 1024
NCORES = 8
BL = B // NCORES  # batches per core
P = 128
NQT = QTL // P  # 4 q tiles
NKT = KTL // P  # 4 k tiles
NDT = D // P    # 8 d tiles
NCH = D // 512  # 2 free-dim chunks of 512
EPS = 1e-5
NEG = -1.0e30

F32 = mybir.dt.float32
BF16 = mybir.dt.bfloat16
F8E4 = mybir.dt.float8e4
AF = mybir.ActivationFunctionType
ALU = mybir.AluOpType
BF = ml_dtypes.bfloat16


def _build(slots, apply1: bool, apply2: bool, b1f: float):
    """slots: list of BL dicts {qup, kup, etp, evp:[per-qt planes]}."""
    nc = bacc.Bacc(None, target_bir_lowering=False)

    QTd = nc.dram_tensor("QTp", [BL, D, QTL], BF16, kind="ExternalInput")
    KTd = nc.dram_tensor("KTp", [BL, D, KTL], BF16, kind="ExternalInput")
    Vd = nc.dram_tensor("Vp", [BL, KTL, D], BF16, kind="ExternalInput")
    Qd = nc.dram_tensor("Qp", [BL, QTL, D], BF16, kind="ExternalInput")
    W1Td = nc.dram_tensor("W1Tp", [D, D], BF16, kind="ExternalInput")
    W2Td = nc.dram_tensor("W2Tp", [D, D], BF16, kind="ExternalInput")
    QSCd = nc.dram_tensor("QSCp", [BL, P, NQT], F32, kind="ExternalInput")
    KBd = nc.dram_tensor("KBp", [BL, KTL], BF16, kind="ExternalInput")
    IDd = nc.dram_tensor("IDp", [P, P], BF16, kind="ExternalInput")
    ONEd = nc.dram_tensor("ONEp", [1, P], BF16, kind="ExternalInput")
    if apply1:
        G1d = nc.dram_tensor("G1p", [D], F32, kind="ExternalInput")
        B1d = nc.dram_tensor("B1p", [D], F32, kind="ExternalInput")
    if apply2:
        G2d = nc.dram_tensor("G2p", [D], F32, kind="ExternalInput")
        B2d = nc.dram_tensor("B2p", [D], F32, kind="ExternalInput")
    OUTd = nc.dram_tensor("OUTp", [BL, QTL, D], F32, kind="ExternalOutput")

    with tile.TileContext(nc) as tc:
        with (
            tc.tile_pool(name="const", bufs=1) as pc,
            tc.tile_pool(name="wts", bufs=1) as pw,
            tc.tile_pool(name="qkin", bufs=2) as pin,
            tc.tile_pool(name="mid", bufs=1) as pmid,
            tc.tile_pool(name="stream", bufs=2) as pst,
            tc.tile_pool(name="small", bufs=2) as psm,
            tc.tile_pool(name="psS", bufs=1, space="PSUM") as psS,
            tc.tile_pool(name="psT", bufs=2, space="PSUM") as psT,
            tc.tile_pool(name="psU", bufs=3, space="PSUM") as psU,
        ):
            # ---- one-time constants ----
            identb = pc.tile([P, P], BF16)
            nc.sync.dma_start(identb, IDd[:, :])
            onesr = pc.tile([1, P], BF16)
            nc.sync.dma_start(onesr, ONEd[:, :])
            zb = pc.tile([P, 1], F32)
            nc.vector.memset(zb, 0.0)
            epsb = pc.tile([P, 1], F32)
            nc.vector.memset(epsb, EPS)
            b1b = pc.tile([P, 1], F32)
            nc.vector.memset(b1b, b1f)
            W1Ts = pw.tile([P, NDT, D], BF16)
            W2Ts = pw.tile([P, NDT, D], BF16)
            if apply1:
                g1t = pc.tile([P, D], F32)
                nc.gpsimd.dma_start(
                    g1t, bass.AP(tensor=G1d, offset=0, ap=[[0, P], [1, D]])
                )
                b1t = pc.tile([P, D], F32)
                nc.gpsimd.dma_start(
                    b1t, bass.AP(tensor=B1d, offset=0, ap=[[0, P], [1, D]])
                )
            if apply2:
                g2t = pc.tile([P, D], F32)
                nc.gpsimd.dma_start(
                    g2t, bass.AP(tensor=G2d, offset=0, ap=[[0, P], [1, D]])
                )
                b2t = pc.tile([P, D], F32)
                nc.gpsimd.dma_start(
                    b2t, bass.AP(tensor=B2d, offset=0, ap=[[0, P], [1, D]])
                )

            state = {}

            def emit_loads(b):
                s = slots[b]
                QTs = pin.tile([P, NDT, QTL], BF16, tag="qts")
                KTs = pin.tile([P, NDT, KTL], BF16, tag="kts")
                qtr = QTd[b].rearrange("(t p) q -> p t q", p=P)
                ktr = KTd[b].rearrange("(t p) k -> p t k", p=P)
                qw = s["qup"] * P
                kw = s["kup"] * P
                qsc = psm.tile([P, NQT], F32, tag="qsc")
                nc.sync.dma_start(qsc, QSCd[b])
                kbr = psm.tile([1, KTL], BF16, tag="kbr")
                nc.sync.dma_start(kbr, KBd[b][None, :])
                npc = 4 if b == 0 else 2
                hd = NDT // npc
                for hh in range(npc):
                    sl = slice(hh * hd, (hh + 1) * hd)
                    nc.sync.dma_start(QTs[:, sl, 0:qw], qtr[:, sl, 0:qw])
                    nc.sync.dma_start(KTs[:, sl, 0:kw], ktr[:, sl, 0:kw])
                if b == 0:
                    nc.scalar.dma_start(
                        W1Ts, W1Td.rearrange("(t p) o -> p t o", p=P)
                    )
                    nc.scalar.dma_start(
                        W2Ts, W2Td.rearrange("(t p) o -> p t o", p=P)
                    )
                Vs = pin.tile([P, NKT, D], BF16, tag="vs")
                vr = Vd[b].rearrange("(t p) d -> p t d", p=P)
                pl = s["etp"]
                nc.sync.dma_start(Vs[:, 0:pl, :], vr[:, 0:pl, :])
                qps = pin.tile([P, NQT, D], BF16, tag="qn")
                qpr = Qd[b].rearrange("(t p) d -> p t d", p=P)
                for hh in range(2):
                    nc.sync.dma_start(
                        qps[:, 2 * hh:2 * hh + 2, :], qpr[:, 2 * hh:2 * hh + 2, :]
                    )

                state[b] = dict(QTs=QTs, KTs=KTs, Vs=Vs, qps=qps, qsc=qsc, kbr=kbr)

            def emit_S(b):
                s = slots[b]
                st = state[b]
                kw = s["kup"] * P
                rowsum = psm.tile([P, NQT], F32, tag="rowsum")
                recip = psm.tile([P, NQT], F32, tag="recip")
                Es = []
                for qt in range(NQT):
                    Sps = psS.tile([P, KTL], F32, tag=f"s{qt % 2}", name="sps")
                    only_kb = qt >= s["qup"]
                    nc.tensor.matmul(
                        Sps, onesr[:, :], st["kbr"][:, :],
                        start=True, stop=only_kb,
                    )
                    if not only_kb:
                        for dt in range(NDT):
                            nc.tensor.matmul(
                                Sps[:, 0:kw],
                                st["QTs"][:, dt, qt * P:(qt + 1) * P],
                                st["KTs"][:, dt, 0:kw],
                                start=False,
                                stop=(dt == NDT - 1),
                            )
                    E = pst.tile([P, KTL], BF16, tag=f"e{qt}", bufs=2, name="E")
                    nc.scalar.activation(
                        E, Sps, AF.Exp,
                        bias=zb[:, :],
                        scale=st["qsc"][:, qt:qt + 1],
                        accum_out=rowsum[:, qt:qt + 1],
                    )
                    Es.append(E)
                    nc.vector.reciprocal(recip[:, qt:qt + 1], rowsum[:, qt:qt + 1])
                    if b == 0 and qt == 0:
                        wsc = psm.tile([P, 4], F32, tag="wsc")
                        nc.vector.tensor_scalar(
                            wsc[:, 0:2], W1Ts[:, 0, 0:4].bitcast(F32),
                            scalar1=1.0, scalar2=None, op0=ALU.mult,
                        )
                        nc.vector.tensor_scalar(
                            wsc[:, 2:4], W2Ts[:, 0, 0:4].bitcast(F32),
                            scalar1=1.0, scalar2=None, op0=ALU.mult,
                        )
                        nc.scalar.dma_start(
                            W1Ts, W1Td.rearrange("(t p) o -> p t o", p=P)
                        )
                        nc.scalar.dma_start(
                            W2Ts, W2Td.rearrange("(t p) o -> p t o", p=P)
                        )
                st["Es"] = Es
                st["recip"] = recip

            def emit_ET(b):
                s = slots[b]
                st = state[b]
                ET = pmid.tile([P, NKT, QTL], BF16, tag="et")
                pl = s["etp"]
                for qt in range(NQT):
                    tps = psT.tile([P, NKT, P], BF16, tag="tr")
                    for kt in range(pl):
                        nc.tensor.transpose(
                            tps[:, kt, :], st["Es"][qt][:, kt * P:(kt + 1) * P],
                            identb,
                        )
                    nc.scalar.copy(
                        ET[:, 0:pl, qt * P:(qt + 1) * P], tps[:, 0:pl, :]
                    )
                st["ET"] = ET

            def emit_EV_ln1(b):
                s = slots[b]
                st = state[b]
                y = pmid.tile([P, NQT, D], BF16, tag="y")
                YT = pmid.tile([P, NDT, QTL], BF16, tag="yt")
                st["y"] = y
                st["YT"] = YT

                def emit_ytr(qt):
                    for half in range(2):
                        tps = psT.tile([P, NKT, P], BF16, tag="tr", name="tps")
                        for j in range(4):
                            dt = half * 4 + j
                            nc.tensor.transpose(
                                tps[:, j, :], y[:, qt, dt * P:(dt + 1) * P],
                                identb,
                            )
                        nc.scalar.copy(
                            YT[:, half * 4:half * 4 + 4, qt * P:(qt + 1) * P],
                            tps,
                        )

                for qt in range(NQT):
                    pl = s["evp"][qt]
                    qres = pst.tile([P, D], F32, tag="big4", bufs=3)
                    for ch in range(NCH):
                        Ups = psU.tile([P, 512], F32, tag="u")
                        for kt in range(pl):
                            nc.tensor.matmul(
                                Ups,
                                st["ET"][:, kt, qt * P:(qt + 1) * P],
                                st["Vs"][:, kt, ch * 512:(ch + 1) * 512],
                                start=(kt == 0),
                                stop=(kt == pl - 1),
                            )
                        nc.vector.scalar_tensor_tensor(
                            qres[:, ch * 512:(ch + 1) * 512],
                            Ups,
                            st["recip"][:, qt:qt + 1],
                            st["qps"][:, qt, ch * 512:(ch + 1) * 512],
                            op0=ALU.mult,
                            op1=ALU.add,
                        )
                    stats = psm.tile([P, NCH, 6], F32, tag="st1")
                    for ch in range(NCH):
                        nc.vector.bn_stats(
                            stats[:, ch, :], qres[:, ch * 512:(ch + 1) * 512]
                        )
                    mv = psm.tile([P, 2], F32, tag="mv1")
                    nc.vector.bn_aggr(mv, stats)
                    rstd = psm.tile([P, 1], F32, tag="std1")
                    nc.scalar.activation(rstd, mv[:, 1:2], AF.Sqrt, bias=epsb[:, :])
                    nc.vector.reciprocal(rstd, rstd)
                    nc.vector.tensor_scalar(
                        y[:, qt, :], qres, scalar1=mv[:, 0:1], scalar2=rstd,
                        op0=ALU.subtract, op1=ALU.mult,
                    )
                    if apply1:
                        yf = y[:, qt, :]
                        nc.vector.tensor_mul(yf, yf, g1t)
                        nc.vector.tensor_add(yf, yf, b1t)
                    if qt >= 2:
                        emit_ytr(qt - 2)
                # remaining yT groups are deferred into the next iteration
                # (after S(b+1)) so the LN1 chain never stalls the PE
                st["ytr_pending"] = [NQT - 2, NQT - 1]
                st["emit_ytr"] = emit_ytr

            def emit_ffn1_half(b, hf):
                st = state[b]
                if hf == 0:
                    st["HT"] = pmid.tile([P, NDT, QTL], BF16, tag="ht", name="HT")
                HT = st["HT"]
                qsl = slice(hf * 256, hf * 256 + 256)
                for ot in range(NDT):
                    Hps = psU.tile([P, 256], F32, tag="u", name="Hps")
                    for dt in range(NDT):
                        nc.tensor.matmul(
                            Hps,
                            W1Ts[:, dt, ot * P:(ot + 1) * P],
                            st["YT"][:, dt, qsl],
                            start=(dt == 0),
                            stop=(dt == NDT - 1),
                        )
                    nc.scalar.activation(
                        HT[:, ot, qsl], Hps, AF.Relu, bias=b1b[:, :]
                    )

            def emit_ffn2_out(b, qts):
                st = state[b]
                for qt in qts:
                    r2 = pst.tile([P, D], F32, tag="big4", bufs=3)
                    for ch in range(NCH):
                        Zps = psU.tile([P, 512], F32, tag="u")
                        for ot in range(NDT):
                            nc.tensor.matmul(
                                Zps,
                                st["HT"][:, ot, qt * P:(qt + 1) * P],
                                W2Ts[:, ot, ch * 512:(ch + 1) * 512],
                                start=(ot == 0),
                                stop=(ot == NDT - 1),
                            )
                        nc.vector.scalar_tensor_tensor(
                            r2[:, ch * 512:(ch + 1) * 512],
                            Zps,
                            1.0,
                            st["y"][:, qt, ch * 512:(ch + 1) * 512],
                            op0=ALU.mult,
                            op1=ALU.add,
                        )
                    stats2 = psm.tile([P, NCH, 6], F32, tag="st2")
                    for ch in range(NCH):
                        nc.vector.bn_stats(
                            stats2[:, ch, :], r2[:, ch * 512:(ch + 1) * 512]
                        )
                    mv2 = psm.tile([P, 2], F32, tag="mv2")
                    nc.vector.bn_aggr(mv2, stats2)
                    rstd2 = psm.tile([P, 1], F32, tag="std2")
                    nc.scalar.activation(
                        rstd2, mv2[:, 1:2], AF.Sqrt, bias=epsb[:, :]
                    )
                    nc.vector.reciprocal(rstd2, rstd2)
                    nmb2 = psm.tile([P, 1], F32, tag="nmb2")
                    nc.vector.scalar_tensor_tensor(
                        nmb2, mv2[:, 0:1], -1.0, rstd2, op0=ALU.mult, op1=ALU.mult
                    )
                    stg = pst.tile([P, D], F32, tag="stg", bufs=2)
                    nc.scalar.activation(
                        stg, r2, AF.Identity, bias=nmb2[:, :], scale=rstd2
                    )
                    if apply2:
                        nc.vector.tensor_mul(stg, stg, g2t)
                        nc.vector.tensor_add(stg, stg, b2t)
                    nc.gpsimd.dma_start(
                        OUTd[b].rearrange("(t p) d -> p t d", p=P)[:, qt, :],
                        stg,
                    )
                if qts[-1] == NQT - 1:
                    del state[b]

            # ---- pipelined emission: attention(b) overlaps FFN(b-1) ----
            # warmup matmuls on (uninitialized) scratch: no DMA dependency,
            # so the PE spins and ramps from t=0 while queues arm and
            # batch-0 inputs stream in
            wscr = pc.tile([P, P], BF16, name="warmsrc")
            nc.vector.memset(wscr, 1.0)
            wps = psT.tile([P, NKT, P], F32, tag="tr", name="warm")
            emit_loads(0)
            for i in range(120):
                nc.tensor.matmul(wps[:, 0:1, :].rearrange("p a b -> p (a b)"),
                                 wscr, wscr, start=True, stop=True)
            for b in range(BL + 1):
                if b < BL:
                    if b + 1 < BL:
                        emit_loads(b + 1)
                    emit_S(b)
                if b >= 1:
                    st = state[b - 1]
                    emit_ffn1_half(b - 1, 0)
                    for qt in st.pop("ytr_pending", []):
                        st["emit_ytr"](qt)
                    emit_ffn1_half(b - 1, 1)
                if b < BL:
                    emit_ET(b)
                if b >= 1:
                    emit_ffn2_out(b - 1, (0, 1))
                    emit_ffn2_out(b - 1, (2, 3))
                if b < BL:
                    emit_EV_ln1(b)

    nc.finalize()
    return nc


def _prepare(Q, K, V, Q_lengths, K_lengths, W1, b1, W2, b2,
             ln1_g, ln1_b, ln2_g, ln2_b):
    Q = np.asarray(Q, dtype=np.float32)
    K = np.asarray(K, dtype=np.float32)
    V = np.asarray(V, dtype=np.float32)
    W1 = np.asarray(W1, dtype=np.float32)
    W2 = np.asarray(W2, dtype=np.float32)
    qlen = np.asarray(Q_lengths).astype(np.int64)
    klen = np.asarray(K_lengths).astype(np.int64)
    g1 = np.asarray(ln1_g, dtype=np.float32)
    b1v = np.asarray(ln1_b, dtype=np.float32)
    g2 = np.asarray(ln2_g, dtype=np.float32)
    b2v = np.asarray(ln2_b, dtype=np.float32)
    b1f = float(np.asarray(b1, dtype=np.float32).reshape(-1)[0])
    # b2 cancels exactly inside LN2 (constant shift removed by mean
    # subtraction), so it is not passed to the device.

    apply1 = not (np.all(g1 == 1.0) and np.all(b1v == 0.0))
    apply2 = not (np.all(g2 == 1.0) and np.all(b2v == 0.0))

    # --- per-batch specialization profile ---
    qup = np.ceil(qlen / P).astype(int)          # S stationary tiles
    kup = np.ceil(klen / P).astype(int)          # S moving width (tiles)
    qfull = (qlen // P).astype(int)              # fully-valid q tiles
    etp = np.where(qfull < NQT, NQT, kup)        # E^T planes needed
    cost = 8 * qup * (kup * P) + (np.minimum(qfull, NQT) * kup
                                  + (NQT - np.minimum(qfull, NQT)) * etp) * NCH * 512

    # Sort batches by cost desc; slot j of every core takes one batch from
    # rank group [8j, 8j+8). The SPMD program specializes slot j to the
    # componentwise max of its group (extra work lands in masked regions).
    order = np.argsort(-cost, kind="stable")
    # rank groups of 8 by descending cost; schedule the cheapest group FIRST
    # (smallest batch-0 inputs -> earliest S start), then the rest by size
    grp_for_slot = [BL - 1] + list(range(BL - 1))
    perm = np.empty(B, dtype=int)  # perm[core*BL + slot] = original batch idx
    for j in range(BL):
        grp = order[grp_for_slot[j] * NCORES:(grp_for_slot[j] + 1) * NCORES]
        for c in range(NCORES):
            perm[c * BL + j] = grp[c]

    slots = []
    for j in range(BL):
        grp = order[grp_for_slot[j] * NCORES:(grp_for_slot[j] + 1) * NCORES]
        squp = int(qup[grp].max())
        skup = int(kup[grp].max())
        setp = int(etp[grp].max())
        sqfull = int(qfull[grp].min())
        evp = [skup if qt < sqfull else setp for qt in range(NQT)]
        slots.append(dict(qup=squp, kup=skup, etp=setp, evp=evp))

    QT = np.ascontiguousarray(Q.transpose(0, 2, 1)).astype(BF)
    KT = np.ascontiguousarray(K.transpose(0, 2, 1)).astype(BF)
    Vb = V.astype(BF)
    Qb = Q.astype(BF)
    W1T = np.ascontiguousarray(W1.T).astype(BF)
    W2T = np.ascontiguousarray(W2.T).astype(BF)

    qmask = (np.arange(QTL)[None, :] < qlen[:, None]).astype(np.float32)
    qsc = (qmask / np.sqrt(np.float32(D))).reshape(B, NQT, P).transpose(0, 2, 1)
    qsc = np.ascontiguousarray(qsc)
    kb = np.where(np.arange(KTL)[None, :] < klen[:, None], 0.0, NEG).astype(BF)
    ident = np.eye(P, dtype=np.float32).astype(BF)
    ones = np.ones((1, P), dtype=np.float32).astype(BF)

    nc = _build(slots, apply1, apply2, b1f)

    in_maps = []
    for c in range(NCORES):
        sel = perm[c * BL:(c + 1) * BL]
        m = {
            "QTp": np.ascontiguousarray(QT[sel]),
            "KTp": np.ascontiguousarray(KT[sel]),
            "Vp": np.ascontiguousarray(Vb[sel]),
            "Qp": np.ascontiguousarray(Qb[sel]),
            "W1Tp": W1T,
            "W2Tp": W2T,
            "QSCp": np.ascontiguousarray(qsc[sel]),
            "KBp": np.ascontiguousarray(kb[sel]),
            "IDp": ident,
            "ONEp": ones,
        }
        if apply1:
            m["G1p"] = g1
            m["B1p"] = b1v
        if apply2:
            m["G2p"] = g2
            m["B2p"] = b2v
        in_maps.append(m)

    return nc, in_maps, perm


def kernel(**inputs):
    nc, in_maps, perm = _prepare(**inputs)
    res = run_bass_kernel_spmd(nc, in_maps, list(range(NCORES)))
    out = np.empty((B, QTL, D), dtype=np.float32)
    for c in range(NCORES):
        out[perm[c * BL:(c + 1) * BL]] = res.results[c]["OUTp"]
    return out
